# revision 31
# baseline (speedup 1.0000x reference)
"""Llama GQA attention block, tensor-parallel over heads across 8 TRN2 NeuronCores.

Contract: kernel(**inputs) takes the FULL inputs of the reference
(x, freq_cos, freq_sin, w_q_w, w_q_b, w_kv_w, w_kv_b, proj_w, proj_b, start_pos)
and returns the FULL output (B, T, N_EMBD) float32.

Sharding: core c owns query heads 4c..4c+3 and KV head c, plus proj rows
c*512..(c+1)*512. Each core computes a partial projection output (fp16); the
host sums the 8 partials and adds proj_b.

v2: all matmul operands in bf16 (f32r runs as two-pass fp32 on the PE —
bf16 halves matmul time), two-phase structure (QKV+RoPE pass with
SBUF-resident Q/K/V, then attention+out-proj), paired heads share score/AV
stationary operands, softmax reciprocal via ScalarE Ln/Exp + PE broadcast
matmul (replaces 4us DVE reciprocal + DRAM-roundtrip broadcast), out-proj
matmuls interleaved into the attention stream as PE filler.
"""

import math
import numpy as np
from contextlib import ExitStack

# Problem constants (hardcoded per the harness contract).
B = 2
T = 2048
E = 4096
D = 128          # head dim
NCORES = 8
HPC = 4          # query heads per core
BT = B * T       # 4096
SQ = 512         # token chunk (matmul moving dim)
ECH = E // 128   # 32 contraction chunks
CPB = T // SQ    # 4 tok chunks per batch
INV_SQRT_D = 1.0 / math.sqrt(D)
HB = 1024        # half-batch token span for the output projection stage


def _build_program():
    import concourse.bass as bass  # noqa: F401
    import concourse.mybir as mybir
    import concourse.tile as tile
    from concourse import bacc

    f32 = mybir.dt.float32
    f32r = mybir.dt.float32r
    bf16 = mybir.dt.bfloat16
    f16 = mybir.dt.float16
    AF = mybir.ActivationFunctionType

    nc = bacc.Bacc("TRN2", target_bir_lowering=False, debug=False)

    xT_d = nc.dram_tensor("xT", [E, BT], bf16, kind="ExternalInput")
    wq_d = nc.dram_tensor("wqT", [E, HPC * D], bf16, kind="ExternalInput")
    wkv_d = nc.dram_tensor("wkvT", [E, 2 * D], bf16, kind="ExternalInput")
    bias_d = nc.dram_tensor("biases", [6, 128], f32, kind="ExternalInput")
    cos_d = nc.dram_tensor("cosE", [128, T], bf16, kind="ExternalInput")
    sin_d = nc.dram_tensor("sinS", [128, T], bf16, kind="ExternalInput")
    mask_d = nc.dram_tensor("maskM", [128, 896], bf16, kind="ExternalInput")
    pjt_d = nc.dram_tensor("projT", [HPC * D, E], bf16, kind="ExternalInput")
    idn_d = nc.dram_tensor("ident", [128, 128], f32, kind="ExternalInput")
    one_d = nc.dram_tensor("onescol", [128, 1], bf16, kind="ExternalInput")
    out_d = nc.dram_tensor("yp", [BT, E], f16, kind="ExternalOutput")

    with tile.TileContext(nc) as tc, ExitStack() as ctx:
        const = ctx.enter_context(tc.tile_pool(name="const", bufs=1))
        wpool = ctx.enter_context(tc.tile_pool(name="wpool", bufs=1))
        big = ctx.enter_context(tc.tile_pool(name="big", bufs=1))
        xpool = ctx.enter_context(tc.tile_pool(name="xpool", bufs=4))
        spool = ctx.enter_context(tc.tile_pool(name="spool", bufs=2))
        ypool = ctx.enter_context(tc.tile_pool(name="ypool", bufs=2))
        drpool = ctx.enter_context(tc.tile_pool(name="drpool", bufs=2,
                                                space="DRAM"))
        psum = ctx.enter_context(tc.tile_pool(name="ps", bufs=1, space="PSUM"))

        # ---- weights / constants resident in SBUF ----
        # wq/wkv load as ONE DMA each (e-outer iteration so early e-slices
        # land first); everything not needed in the first chunk is deferred
        # into the chunk-0 emission to keep the sync DMA queue clear.
        wq_sb = wpool.tile([128, ECH, HPC * D], bf16, tag="wq")
        wkv_sb = wpool.tile([128, ECH, 2 * D], bf16, tag="wkv")
        # contiguous per-e loads; wq staggered into the first chunk's e-loop
        # on the sync queue, wkv whole on the ScalarE HWDGE queue (idle then)
        for e in range(3):
            nc.sync.dma_start(wq_sb[:, e, :], wq_d[e * 128:(e + 1) * 128, :])
        for e in range(ECH):
            nc.scalar.dma_start(wkv_sb[:, e, :],
                                wkv_d[e * 128:(e + 1) * 128, :])
        bias_sb = const.tile([128, 6], f32, tag="bias")
        cos_sb = const.tile([128, T], bf16, tag="cos")
        sin_sb = const.tile([128, T], bf16, tag="sin")
        mask_sb = const.tile([128, 896], bf16, tag="mask")
        idn_sb = const.tile([128, 128], f32, tag="idn")
        ones_sb = const.tile([128, 1], bf16, tag="ones")
        pjt_sb = wpool.tile([128, HPC, E], bf16, tag="pjt")

        def load_consts():
            nc.scalar.dma_start(bias_sb[:], bias_d.rearrange("r p -> p r"))
            nc.scalar.dma_start(cos_sb[:], cos_d[:, :])
            nc.scalar.dma_start(sin_sb[:], sin_d[:, :])
            nc.scalar.dma_start(idn_sb[:], idn_d[:, :])
            nc.scalar.dma_start(mask_sb[:], mask_d[:, :])
            nc.scalar.dma_start(ones_sb[:], one_d[:, :])

        # big SBUF-resident intermediates (bf16): rotated Q, rotated K, V^T
        qrot = big.tile([128, HPC, BT], bf16, tag="qrot")   # [d, h, tok]
        rotK = big.tile([128, BT], bf16, tag="rotK")        # [d, tok]
        vbufT = big.tile([128, BT], bf16, tag="vbuf")       # [tok%128, kc*128+d]

        ps_tag = [f"b{i}" for i in range(8)]
        tp_alt = 0      # V-transpose bank alternator (b6/b7)
        po_alt = 0      # out-proj bank alternator (b0/b1)

        # ---------------- pass 1: QKV projection + rope ----------------
        # V transposes are deferred into the NEXT chunk's matmul stream so
        # the PE never waits on the ScalarE eviction chain at chunk edges.
        pending_v = []

        def flush_v(base=6):
            nonlocal tp_alt
            while pending_v:
                vraw_p, gcol_p = pending_v.pop(0)
                for t4 in range(4):
                    tp = psum.tile([128, 128], f32, tag=ps_tag[base + tp_alt],
                                   name=f"tp_{gcol_p}_{t4}")
                    tp_alt ^= 1
                    nc.tensor.transpose(
                        tp[:], vraw_p[:, t4 * 128:(t4 + 1) * 128], idn_sb[:])
                    nc.scalar.copy(vbufT[:, gcol_p + t4 * 128:
                                         gcol_p + (t4 + 1) * 128], tp[:])

        for b in range(B):
            for j in range(CPB):
                gcol = b * T + j * SQ
                tcol = j * SQ
                acc = [psum.tile([128, SQ], f32, tag=ps_tag[i],
                                 name=f"acc{i}_{b}_{j}") for i in range(6)]
                for e in range(ECH):
                    xt = xpool.tile([128, SQ], bf16, tag="xt")
                    nc.sync.dma_start(xt[:], xT_d[e * 128:(e + 1) * 128,
                                                  gcol:gcol + SQ])
                    st, sp = (e == 0), (e == ECH - 1)
                    for h in range(HPC):
                        nc.tensor.matmul(
                            acc[h][:], wq_sb[:, e, h * D:(h + 1) * D],
                            xt[:], start=st, stop=sp)
                    nc.tensor.matmul(acc[4][:], wkv_sb[:, e, 0:D],
                                     xt[:], start=st, stop=sp)
                    nc.tensor.matmul(acc[5][:], wkv_sb[:, e, D:2 * D],
                                     xt[:], start=st, stop=sp)
                    if e == 10:
                        flush_v()
                    if b == 0 and j == 0:
                        if e == 6:
                            load_consts()
                        if e < ECH - 3:
                            eng = nc.sync if e + 3 <= 10 else nc.scalar
                            eng.dma_start(
                                wq_sb[:, e + 3, :],
                                wq_d[(e + 3) * 128:(e + 4) * 128, :])
                    if b == 1 and j == 2 and e == 6:
                        nc.scalar.dma_start(
                            pjt_sb[:],
                            pjt_d.rearrange("(h p) o -> p h o", p=128))

                def rope(ps, bias_col, out_ap, nm):
                    raw = spool.tile([128, SQ], bf16, tag="raw", bufs=3,
                                     name=f"raw_{nm}")
                    nc.scalar.activation(raw[:], ps[:], AF.Identity,
                                         bias=bias_sb[:, bias_col:bias_col + 1])
                    sw = spool.tile([128, SQ], bf16, tag="sw", bufs=3,
                                    name=f"sw_{nm}")
                    raw3 = raw.rearrange("(a two) t -> a two t", two=2)
                    sw3 = sw.rearrange("(a two) t -> a two t", two=2)
                    nc.scalar.dma_start(sw3[:, 1, :], raw3[:, 0, :])
                    nc.scalar.dma_start(sw3[:, 0, :], raw3[:, 1, :])
                    tmp = spool.tile([128, SQ], bf16, tag="rtmp", bufs=3,
                                     name=f"rtmp_{nm}")
                    nc.vector.tensor_mul(tmp[:], raw[:],
                                         cos_sb[:, tcol:tcol + SQ])
                    nc.vector.tensor_mul(sw[:], sw[:],
                                         sin_sb[:, tcol:tcol + SQ])
                    nc.vector.tensor_add(out_ap, tmp[:], sw[:])

                for h in range(HPC):
                    rope(acc[h], h, qrot[:, h, gcol:gcol + SQ], f"{b}_{j}_{h}")
                rope(acc[4], 4, rotK[:, gcol:gcol + SQ], f"{b}_{j}_k")

                vraw = spool.tile([128, SQ], f32, tag="vraw", bufs=2,
                                  name=f"vraw_{b}_{j}")
                nc.scalar.activation(vraw[:], acc[5][:], AF.Identity,
                                     bias=bias_sb[:, 5:6])
                pending_v.append((vraw, gcol))
        # the last chunk's V transposes are emitted early in pass 2 (on the
        # then-idle out-proj banks) so pass 2 starts immediately

        # ------------- pass 2: attention + output projection -------------
        pend = []   # pending out-proj groups from the previous half-batch

        def po_group(yts_t, grow, oc, ts8):
            nonlocal po_alt
            po_ps = psum.tile([128, SQ], f32, tag=ps_tag[po_alt],
                              name=f"pops_{grow}_{oc}_{ts8}")
            for h in range(HPC):
                nc.tensor.matmul(
                    po_ps[:], yts_t[:, h, ts8 * 128:(ts8 + 1) * 128],
                    pjt_sb[:, h, oc * SQ:(oc + 1) * SQ],
                    start=(h == 0), stop=(h == HPC - 1))
            po = spool.tile([128, SQ], f16, tag="po", bufs=4)
            # alternate eviction engine to balance ScalarE/VectorE load
            if po_alt:
                nc.vector.tensor_copy(po[:], po_ps[:])
            else:
                nc.scalar.copy(po[:], po_ps[:])
            po_alt ^= 1
            nc.sync.dma_start(
                out_d[grow + ts8 * 128:grow + (ts8 + 1) * 128,
                      oc * SQ:(oc + 1) * SQ], po[:])

        def pop_pend(n, reserve=0):
            for _ in range(n):
                if len(pend) <= reserve:
                    return
                pend.pop(0)()

        for b in range(B):
            for hh in range(2):
                yts = ypool.tile([128, HPC, HB], bf16, tag="yts",
                                 name=f"yts_{b}_{hh}")
                # unnormalized attention outputs + row-sums, staged in SBUF
                # so PSUM banks free immediately; softmax normalization is
                # batched per half-batch (avoids Exp<->Ln table thrashing).
                yty = ypool.tile([128, 8, SQ], bf16, tag="yty", bufs=1,
                                 name=f"yty_{b}_{hh}")
                sums8 = ypool.tile([64, SQ], f32, tag="sums", bufs=1,
                                   name=f"sums_{b}_{hh}")
                for jj in range(2):
                    j = hh * 2 + jj
                    last_jj = (b == 1 and hh == 1 and jj == 1)
                    gcol = b * T + j * SQ
                    nkc = 4 * j + 4
                    for pr in range(2):          # head pairs (2h per pass)
                        hs = (2 * pr, 2 * pr + 1)
                        yt_ps = [psum.tile([128, SQ], f32, tag=ps_tag[4 + i],
                                           name=f"yt_{b}_{j}_{h}")
                                 for i, h in enumerate(hs)]
                        sm_ps = [psum.tile([1, SQ], f32, tag=ps_tag[2 + i],
                                           name=f"sm_{b}_{j}_{h}")
                                 for i, h in enumerate(hs)]
                        prev_es = None

                        def emit_av(kc, es_pair):
                            st, sp = (kc == 0), (kc == nkc - 1)
                            koff = b * T + kc * 128
                            for i in range(2):
                                nc.tensor.matmul(
                                    yt_ps[i][:], vbufT[:, koff:koff + 128],
                                    es_pair[i][:], start=st, stop=sp)
                            for i in range(2):
                                nc.tensor.matmul(
                                    sm_ps[i][:], ones_sb[:], es_pair[i][:],
                                    start=st, stop=sp)

                        for kc in range(nkc):
                            koff = b * T + kc * 128
                            s_ps = []
                            for i, h in enumerate(hs):
                                sp_t = psum.tile(
                                    [128, SQ], f32, tag=ps_tag[6 + i],
                                    name=f"s_{b}_{j}_{kc}_{h}")
                                nc.tensor.matmul(
                                    sp_t[:], rotK[:, koff:koff + 128],
                                    qrot[:, h, gcol:gcol + SQ],
                                    start=True, stop=True)
                                s_ps.append(sp_t)
                            es_pair = []
                            for i, h in enumerate(hs):
                                es_t = spool.tile(
                                    [128, SQ], bf16, tag=f"es{i}", bufs=3,
                                    name=f"es_{b}_{j}_{kc}_{h}")
                                nc.scalar.activation(es_t[:], s_ps[i][:],
                                                     AF.Exp, scale=INV_SQRT_D)
                                if kc >= nkc - 4:
                                    off = (3 - (kc - (nkc - 4))) * 128
                                    nc.vector.tensor_mul(
                                        es_t[:], es_t[:],
                                        mask_sb[:, off:off + SQ])
                                es_pair.append(es_t)
                            if prev_es is not None:
                                emit_av(kc - 1, prev_es)
                            prev_es = es_pair
                            pop_pend(2, reserve=(14 if last_jj else 0))
                        emit_av(nkc - 1, prev_es)

                        # stage results out of PSUM right away (frees banks);
                        # sums hop via a partition-0 tile + DMA into row sidx
                        for i, h in enumerate(hs):
                            sidx = jj * 4 + pr * 2 + i
                            srow = jj * 32 + pr * 2 + i
                            stg = spool.tile([1, SQ], f32, tag="stg", bufs=2,
                                             name=f"stg_{b}_{j}_{h}")
                            nc.scalar.copy(stg[:], sm_ps[i][:])
                            nc.scalar.dma_start(sums8[srow:srow + 1, :],
                                                stg[:])
                            nc.vector.tensor_copy(yty[:, sidx, :],
                                                  yt_ps[i][:])
                        pop_pend(2)
                        if b == 0 and hh == 0 and jj == 0 and pr == 0:
                            flush_v(base=0)

                    # per-j softmax normalization: r = exp(-ln(sum)), one
                    # table switch each way, broadcast via DRAM roundtrip;
                    # the multiplies and this quarter's out-proj go into the
                    # filler stream (popped during subsequent attention).
                    r0 = jj * 32
                    rr = spool.tile([4, SQ], bf16, tag="rr", bufs=2,
                                    name=f"rr_{b}_{hh}_{jj}")
                    with nc.allow_low_precision(reason="bf16 softmax recip"):
                        nc.vector.reciprocal(rr[:], sums8[r0:r0 + 4, :])
                    dr = drpool.tile([4, SQ], bf16, tag="dr",
                                     name=f"dr_{b}_{hh}_{jj}")
                    nc.sync.dma_start(dr[:], rr[:])

                    def norm_mul(yts_t, yty_t, dr_t, jj_, pr, i, bb=b, hh_=hh):
                        sidx = jj_ * 4 + pr * 2 + i
                        h = pr * 2 + i
                        rb = spool.tile([128, SQ], bf16, tag="rb", bufs=3,
                                        name=f"rb_{bb}_{hh_}_{sidx}")
                        nc.sync.dma_start(
                            rb[:], dr_t[pr * 2 + i:pr * 2 + i + 1, :]
                            .to_broadcast((128, SQ)))
                        nc.vector.tensor_mul(
                            yts_t[:, h, jj_ * SQ:(jj_ + 1) * SQ],
                            yty_t[:, sidx, :], rb[:])

                    for pr in range(2):
                        for i in range(2):
                            pend.append(
                                lambda y=yts, yy=yty, dd=dr, a=jj, p=pr, q=i:
                                norm_mul(y, yy, dd, a, p, q))
                    grow = b * T + hh * HB
                    for oc in range(8):
                        for ts8 in range(jj * 4, jj * 4 + 4):
                            pend.append(
                                lambda y=yts, g=grow, o=oc, t=ts8:
                                po_group(y, g, o, t))
        pop_pend(len(pend))

    nc.compile()
    return nc


_PROG = None


def kernel(x, freq_cos, freq_sin, w_q_w, w_q_b, w_kv_w, w_kv_b, proj_w, proj_b,
           start_pos=0, **_unused):
    global _PROG
    import ml_dtypes
    from concourse.bass_utils import run_bass_kernel_spmd

    bf16 = ml_dtypes.bfloat16

    x = np.asarray(x, np.float32)
    freq_cos = np.asarray(freq_cos, np.float32)
    freq_sin = np.asarray(freq_sin, np.float32)
    w_q_w = np.asarray(w_q_w, np.float32)
    w_q_b = np.asarray(w_q_b, np.float32)
    w_kv_w = np.asarray(w_kv_w, np.float32)
    w_kv_b = np.asarray(w_kv_b, np.float32)
    proj_w = np.asarray(proj_w, np.float32)
    proj_b = np.asarray(proj_b, np.float32)

    xT = np.ascontiguousarray(x.reshape(BT, E).T).astype(bf16)

    cosE = np.repeat(freq_cos.T, 2, axis=0).astype(np.float32)        # [128, T]
    sinE = np.repeat(freq_sin.T, 2, axis=0).astype(np.float32)
    sinS = sinE.copy()
    sinS[0::2, :] *= -1.0                                             # even rows -sin
    cosE = cosE.astype(bf16)
    sinS = sinS.astype(bf16)

    kp = np.arange(128)[:, None]
    cc = np.arange(896)[None, :]
    maskM = (cc >= kp + 384).astype(bf16)

    ident = np.eye(128, dtype=np.float32)

    if _PROG is None:
        _PROG = _build_program()

    in_maps = []
    for c in range(NCORES):
        wq_c = np.ascontiguousarray(
            w_q_w[c * 512:(c + 1) * 512, :].T).astype(bf16)            # [E, 512]
        kT = w_kv_w[c * D:(c + 1) * D, :].T                            # [E, 128]
        vT = w_kv_w[8 * D + c * D:8 * D + (c + 1) * D, :].T
        wkv_c = np.ascontiguousarray(
            np.concatenate([kT, vT], axis=1)).astype(bf16)             # [E, 256]
        biases = np.zeros((6, 128), np.float32)
        biases[0:4, :] = w_q_b[c * 512:(c + 1) * 512].reshape(4, 128)
        biases[4, :] = w_kv_b[c * D:(c + 1) * D]
        biases[5, :] = w_kv_b[8 * D + c * D:8 * D + (c + 1) * D]
        pjt_c = np.ascontiguousarray(
            proj_w[:, c * 512:(c + 1) * 512].T).astype(bf16)           # [512, E]
        in_maps.append({
            "xT": xT, "wqT": wq_c, "wkvT": wkv_c, "biases": biases,
            "cosE": cosE, "sinS": sinS, "maskM": maskM, "projT": pjt_c,
            "ident": ident, "onescol": np.ones((128, 1), bf16),
        })

    res = run_bass_kernel_spmd(_PROG, in_maps, core_ids=list(range(NCORES)))
    out = np.zeros((BT, E), np.float32)
    for c in range(NCORES):
        out += res.results[c]["yp"].astype(np.float32)
    out = out + proj_b[None, :].astype(np.float32)
    return out.reshape(B, T, E).astype(np.float32)


# revision 32
# speedup vs baseline: 1.0568x; 1.0568x over previous
"""Llama GQA attention block, tensor-parallel over heads across 8 TRN2 NeuronCores.

Contract: kernel(**inputs) takes the FULL inputs of the reference
(x, freq_cos, freq_sin, w_q_w, w_q_b, w_kv_w, w_kv_b, proj_w, proj_b, start_pos)
and returns the FULL output (B, T, N_EMBD) float32.

Sharding: core c owns query heads 4c..4c+3 and KV head c, plus proj rows
c*512..(c+1)*512. Each core computes a partial projection output (fp16); the
host sums the 8 partials and adds proj_b.

v2: all matmul operands in bf16 (f32r runs as two-pass fp32 on the PE —
bf16 halves matmul time), two-phase structure (QKV+RoPE pass with
SBUF-resident Q/K/V, then attention+out-proj), paired heads share score/AV
stationary operands, softmax reciprocal via ScalarE Ln/Exp + PE broadcast
matmul (replaces 4us DVE reciprocal + DRAM-roundtrip broadcast), out-proj
matmuls interleaved into the attention stream as PE filler.
"""

import math
import numpy as np
from contextlib import ExitStack

# Problem constants (hardcoded per the harness contract).
B = 2
T = 2048
E = 4096
D = 128          # head dim
NCORES = 8
HPC = 4          # query heads per core
BT = B * T       # 4096
SQ = 512         # token chunk (matmul moving dim)
ECH = E // 128   # 32 contraction chunks
CPB = T // SQ    # 4 tok chunks per batch
INV_SQRT_D = 1.0 / math.sqrt(D)
HB = 1024        # half-batch token span for the output projection stage


def _build_program():
    import concourse.bass as bass  # noqa: F401
    import concourse.mybir as mybir
    import concourse.tile as tile
    from concourse import bacc

    f32 = mybir.dt.float32
    f32r = mybir.dt.float32r
    bf16 = mybir.dt.bfloat16
    f16 = mybir.dt.float16
    AF = mybir.ActivationFunctionType

    nc = bacc.Bacc("TRN2", target_bir_lowering=False, debug=False)

    xT_d = nc.dram_tensor("xT", [E, BT], bf16, kind="ExternalInput")
    wq_d = nc.dram_tensor("wqT", [E, HPC * D], bf16, kind="ExternalInput")
    wkv_d = nc.dram_tensor("wkvT", [E, 2 * D], bf16, kind="ExternalInput")
    bias_d = nc.dram_tensor("biases", [6, 128], f32, kind="ExternalInput")
    cos_d = nc.dram_tensor("cosE", [128, T], bf16, kind="ExternalInput")
    sin_d = nc.dram_tensor("sinS", [128, T], bf16, kind="ExternalInput")
    mask_d = nc.dram_tensor("maskM", [128, 896], bf16, kind="ExternalInput")
    pjt_d = nc.dram_tensor("projT", [HPC * D, E], bf16, kind="ExternalInput")
    idn_d = nc.dram_tensor("ident", [128, 128], f32, kind="ExternalInput")
    one_d = nc.dram_tensor("onescol", [128, 1], bf16, kind="ExternalInput")
    out_d = nc.dram_tensor("yp", [BT, E], f16, kind="ExternalOutput")

    with tile.TileContext(nc) as tc, ExitStack() as ctx:
        const = ctx.enter_context(tc.tile_pool(name="const", bufs=1))
        wpool = ctx.enter_context(tc.tile_pool(name="wpool", bufs=1))
        big = ctx.enter_context(tc.tile_pool(name="big", bufs=1))
        xpool = ctx.enter_context(tc.tile_pool(name="xpool", bufs=4))
        spool = ctx.enter_context(tc.tile_pool(name="spool", bufs=2))
        ypool = ctx.enter_context(tc.tile_pool(name="ypool", bufs=2))
        drpool = ctx.enter_context(tc.tile_pool(name="drpool", bufs=2,
                                                space="DRAM"))
        psum = ctx.enter_context(tc.tile_pool(name="ps", bufs=1, space="PSUM"))

        # ---- weights / constants resident in SBUF ----
        # wq/wkv load as ONE DMA each (e-outer iteration so early e-slices
        # land first); everything not needed in the first chunk is deferred
        # into the chunk-0 emission to keep the sync DMA queue clear.
        wq_sb = wpool.tile([128, ECH, HPC * D], bf16, tag="wq")
        wkv_sb = wpool.tile([128, ECH, 2 * D], bf16, tag="wkv")
        # contiguous per-e loads; wq staggered into the first chunk's e-loop
        # on the sync queue, wkv whole on the ScalarE HWDGE queue (idle then)
        for e in range(3):
            nc.sync.dma_start(wq_sb[:, e, :], wq_d[e * 128:(e + 1) * 128, :])
        for e in range(ECH):
            nc.scalar.dma_start(wkv_sb[:, e, :],
                                wkv_d[e * 128:(e + 1) * 128, :])
        bias_sb = const.tile([128, 6], f32, tag="bias")
        cos_sb = const.tile([128, T], bf16, tag="cos")
        sin_sb = const.tile([128, T], bf16, tag="sin")
        mask_sb = const.tile([128, 896], bf16, tag="mask")
        idn_sb = const.tile([128, 128], f32, tag="idn")
        ones_sb = const.tile([128, 1], bf16, tag="ones")
        oneb_sb = const.tile([128, 128], bf16, tag="oneb")
        nc.any.memset(oneb_sb[:], 1.0)
        pjt_sb = wpool.tile([128, HPC, E], bf16, tag="pjt")

        def load_consts():
            nc.scalar.dma_start(bias_sb[:], bias_d.rearrange("r p -> p r"))
            nc.scalar.dma_start(cos_sb[:], cos_d[:, :])
            nc.scalar.dma_start(sin_sb[:], sin_d[:, :])
            nc.scalar.dma_start(idn_sb[:], idn_d[:, :])
            nc.scalar.dma_start(mask_sb[:], mask_d[:, :])
            nc.scalar.dma_start(ones_sb[:], one_d[:, :])

        # big SBUF-resident intermediates (bf16): rotated Q, rotated K, V^T
        qrot = big.tile([128, HPC, BT], bf16, tag="qrot")   # [d, h, tok]
        rotK = big.tile([128, BT], bf16, tag="rotK")        # [d, tok]
        vbufT = big.tile([128, BT], bf16, tag="vbuf")       # [tok%128, kc*128+d]

        ps_tag = [f"b{i}" for i in range(8)]
        tp_alt = 0      # V-transpose bank alternator (b6/b7)
        po_alt = 0      # out-proj bank alternator (b0/b1)

        # ---------------- pass 1: QKV projection + rope ----------------
        # V transposes are deferred into the NEXT chunk's matmul stream so
        # the PE never waits on the ScalarE eviction chain at chunk edges.
        pending_v = []

        def flush_v(base=6):
            nonlocal tp_alt
            while pending_v:
                vraw_p, gcol_p = pending_v.pop(0)
                for t4 in range(4):
                    tp = psum.tile([128, 128], f32, tag=ps_tag[base + tp_alt],
                                   name=f"tp_{gcol_p}_{t4}")
                    tp_alt ^= 1
                    nc.tensor.transpose(
                        tp[:], vraw_p[:, t4 * 128:(t4 + 1) * 128], idn_sb[:])
                    nc.scalar.copy(vbufT[:, gcol_p + t4 * 128:
                                         gcol_p + (t4 + 1) * 128], tp[:])

        for b in range(B):
            for j in range(CPB):
                gcol = b * T + j * SQ
                tcol = j * SQ
                acc = [psum.tile([128, SQ], f32, tag=ps_tag[i],
                                 name=f"acc{i}_{b}_{j}") for i in range(6)]
                for e in range(ECH):
                    xt = xpool.tile([128, SQ], bf16, tag="xt")
                    nc.sync.dma_start(xt[:], xT_d[e * 128:(e + 1) * 128,
                                                  gcol:gcol + SQ])
                    st, sp = (e == 0), (e == ECH - 1)
                    for h in range(HPC):
                        nc.tensor.matmul(
                            acc[h][:], wq_sb[:, e, h * D:(h + 1) * D],
                            xt[:], start=st, stop=sp)
                    nc.tensor.matmul(acc[4][:], wkv_sb[:, e, 0:D],
                                     xt[:], start=st, stop=sp)
                    nc.tensor.matmul(acc[5][:], wkv_sb[:, e, D:2 * D],
                                     xt[:], start=st, stop=sp)
                    if e == 10:
                        flush_v()
                    if b == 0 and j == 0:
                        if e == 6:
                            load_consts()
                        if e < ECH - 3:
                            eng = nc.sync if e + 3 <= 10 else nc.scalar
                            eng.dma_start(
                                wq_sb[:, e + 3, :],
                                wq_d[(e + 3) * 128:(e + 4) * 128, :])
                    if b == 1 and j == 2 and e == 6:
                        nc.scalar.dma_start(
                            pjt_sb[:],
                            pjt_d.rearrange("(h p) o -> p h o", p=128))

                def rope(ps, bias_col, out_ap, nm):
                    raw = spool.tile([128, SQ], bf16, tag="raw", bufs=3,
                                     name=f"raw_{nm}")
                    nc.scalar.activation(raw[:], ps[:], AF.Identity,
                                         bias=bias_sb[:, bias_col:bias_col + 1])
                    sw = spool.tile([128, SQ], bf16, tag="sw", bufs=3,
                                    name=f"sw_{nm}")
                    raw3 = raw.rearrange("(a two) t -> a two t", two=2)
                    sw3 = sw.rearrange("(a two) t -> a two t", two=2)
                    nc.scalar.dma_start(sw3[:, 1, :], raw3[:, 0, :])
                    nc.scalar.dma_start(sw3[:, 0, :], raw3[:, 1, :])
                    tmp = spool.tile([128, SQ], bf16, tag="rtmp", bufs=3,
                                     name=f"rtmp_{nm}")
                    nc.vector.tensor_mul(tmp[:], raw[:],
                                         cos_sb[:, tcol:tcol + SQ])
                    nc.vector.tensor_mul(sw[:], sw[:],
                                         sin_sb[:, tcol:tcol + SQ])
                    nc.vector.tensor_add(out_ap, tmp[:], sw[:])

                for h in range(HPC):
                    rope(acc[h], h, qrot[:, h, gcol:gcol + SQ], f"{b}_{j}_{h}")
                rope(acc[4], 4, rotK[:, gcol:gcol + SQ], f"{b}_{j}_k")

                vraw = spool.tile([128, SQ], f32, tag="vraw", bufs=2,
                                  name=f"vraw_{b}_{j}")
                nc.scalar.activation(vraw[:], acc[5][:], AF.Identity,
                                     bias=bias_sb[:, 5:6])
                pending_v.append((vraw, gcol))
        # the last chunk's V transposes are emitted early in pass 2 (on the
        # then-idle out-proj banks) so pass 2 starts immediately

        # ------------- pass 2: attention + output projection -------------
        pend = []   # pending out-proj groups from the previous half-batch

        def po_group(yts_t, grow, oc, ts8):
            nonlocal po_alt
            po_ps = psum.tile([128, SQ], f32, tag=ps_tag[po_alt],
                              name=f"pops_{grow}_{oc}_{ts8}")
            for h in range(HPC):
                nc.tensor.matmul(
                    po_ps[:], yts_t[:, h, ts8 * 128:(ts8 + 1) * 128],
                    pjt_sb[:, h, oc * SQ:(oc + 1) * SQ],
                    start=(h == 0), stop=(h == HPC - 1))
            po = spool.tile([128, SQ], f16, tag="po", bufs=4)
            # alternate eviction engine to balance ScalarE/VectorE load
            if po_alt:
                nc.vector.tensor_copy(po[:], po_ps[:])
            else:
                nc.scalar.copy(po[:], po_ps[:])
            po_alt ^= 1
            nc.sync.dma_start(
                out_d[grow + ts8 * 128:grow + (ts8 + 1) * 128,
                      oc * SQ:(oc + 1) * SQ], po[:])

        def pop_pend(n, reserve=0):
            for _ in range(n):
                if len(pend) <= reserve:
                    return
                pend.pop(0)()

        for b in range(B):
            for hh in range(2):
                yts = ypool.tile([128, HPC, HB], bf16, tag="yts",
                                 name=f"yts_{b}_{hh}")
                # unnormalized attention outputs + row-sums, staged in SBUF
                # so PSUM banks free immediately; softmax normalization is
                # batched per half-batch (avoids Exp<->Ln table thrashing).
                yty = ypool.tile([128, 8, SQ], bf16, tag="yty", bufs=1,
                                 name=f"yty_{b}_{hh}")
                sums8 = ypool.tile([64, SQ], f32, tag="sums", bufs=1,
                                   name=f"sums_{b}_{hh}")
                for jj in range(2):
                    j = hh * 2 + jj
                    last_jj = (b == 1 and hh == 1 and jj == 1)
                    gcol = b * T + j * SQ
                    nkc = 4 * j + 4
                    for pr in range(2):          # head pairs (2h per pass)
                        hs = (2 * pr, 2 * pr + 1)
                        yt_ps = [psum.tile([128, SQ], f32, tag=ps_tag[4 + i],
                                           name=f"yt_{b}_{j}_{h}")
                                 for i, h in enumerate(hs)]
                        sm_ps = [psum.tile([128, SQ], f32,
                                           tag=ps_tag[2 + i],
                                           name=f"sm_{b}_{j}_{h}")
                                 for i, h in enumerate(hs)]
                        prev_es = None

                        def emit_av(kc, es_pair):
                            st, sp = (kc == 0), (kc == nkc - 1)
                            koff = b * T + kc * 128
                            for i in range(2):
                                nc.tensor.matmul(
                                    yt_ps[i][:], vbufT[:, koff:koff + 128],
                                    es_pair[i][:], start=st, stop=sp)
                            for i in range(2):
                                nc.tensor.matmul(
                                    sm_ps[i][:], oneb_sb[:], es_pair[i][:],
                                    start=st, stop=sp)

                        for kc in range(nkc):
                            koff = b * T + kc * 128
                            s_ps = []
                            for i, h in enumerate(hs):
                                sp_t = psum.tile(
                                    [128, SQ], f32, tag=ps_tag[6 + i],
                                    name=f"s_{b}_{j}_{kc}_{h}")
                                nc.tensor.matmul(
                                    sp_t[:], rotK[:, koff:koff + 128],
                                    qrot[:, h, gcol:gcol + SQ],
                                    start=True, stop=True)
                                s_ps.append(sp_t)
                            es_pair = []
                            for i, h in enumerate(hs):
                                es_t = spool.tile(
                                    [128, SQ], bf16, tag=f"es{i}", bufs=3,
                                    name=f"es_{b}_{j}_{kc}_{h}")
                                nc.scalar.activation(es_t[:], s_ps[i][:],
                                                     AF.Exp, scale=INV_SQRT_D)
                                if kc >= nkc - 4:
                                    off = (3 - (kc - (nkc - 4))) * 128
                                    nc.vector.tensor_mul(
                                        es_t[:], es_t[:],
                                        mask_sb[:, off:off + SQ])
                                es_pair.append(es_t)
                            if prev_es is not None:
                                emit_av(kc - 1, prev_es)
                            prev_es = es_pair
                            pop_pend(2, reserve=(14 if last_jj else 0))
                        emit_av(nkc - 1, prev_es)

                        # stage results out of PSUM right away (frees banks);
                        # sums hop via a partition-0 tile + DMA into row sidx
                        for i, h in enumerate(hs):
                            sidx = jj * 4 + pr * 2 + i
                            srow = jj * 32 + pr * 2 + i
                            stg = spool.tile([1, SQ], f32, tag="stg", bufs=2,
                                             name=f"stg_{b}_{j}_{h}")
                            nc.scalar.copy(stg[:], sm_ps[i][0:1, :])
                            nc.scalar.dma_start(sums8[srow:srow + 1, :],
                                                stg[:])
                            nc.vector.tensor_copy(yty[:, sidx, :],
                                                  yt_ps[i][:])
                        pop_pend(2)
                        if b == 0 and hh == 0 and jj == 0 and pr == 0:
                            flush_v(base=0)

                    # per-j softmax normalization: r = exp(-ln(sum)), one
                    # table switch each way, broadcast via DRAM roundtrip;
                    # the multiplies and this quarter's out-proj go into the
                    # filler stream (popped during subsequent attention).
                    r0 = jj * 32
                    rr = spool.tile([4, SQ], bf16, tag="rr", bufs=2,
                                    name=f"rr_{b}_{hh}_{jj}")
                    with nc.allow_low_precision(reason="bf16 softmax recip"):
                        nc.vector.reciprocal(rr[:], sums8[r0:r0 + 4, :])
                    dr = drpool.tile([4, SQ], bf16, tag="dr",
                                     name=f"dr_{b}_{hh}_{jj}")
                    nc.sync.dma_start(dr[:], rr[:])

                    def norm_mul(yts_t, yty_t, dr_t, jj_, pr, i, bb=b, hh_=hh):
                        sidx = jj_ * 4 + pr * 2 + i
                        h = pr * 2 + i
                        rb = spool.tile([128, SQ], bf16, tag="rb", bufs=3,
                                        name=f"rb_{bb}_{hh_}_{sidx}")
                        nc.sync.dma_start(
                            rb[:], dr_t[pr * 2 + i:pr * 2 + i + 1, :]
                            .to_broadcast((128, SQ)))
                        nc.vector.tensor_mul(
                            yts_t[:, h, jj_ * SQ:(jj_ + 1) * SQ],
                            yty_t[:, sidx, :], rb[:])

                    for pr in range(2):
                        for i in range(2):
                            pend.append(
                                lambda y=yts, yy=yty, dd=dr, a=jj, p=pr, q=i:
                                norm_mul(y, yy, dd, a, p, q))
                    grow = b * T + hh * HB
                    for oc in range(8):
                        for ts8 in range(jj * 4, jj * 4 + 4):
                            pend.append(
                                lambda y=yts, g=grow, o=oc, t=ts8:
                                po_group(y, g, o, t))
        pop_pend(len(pend))

    nc.compile()
    return nc


_PROG = None


def kernel(x, freq_cos, freq_sin, w_q_w, w_q_b, w_kv_w, w_kv_b, proj_w, proj_b,
           start_pos=0, **_unused):
    global _PROG
    import ml_dtypes
    from concourse.bass_utils import run_bass_kernel_spmd

    bf16 = ml_dtypes.bfloat16

    x = np.asarray(x, np.float32)
    freq_cos = np.asarray(freq_cos, np.float32)
    freq_sin = np.asarray(freq_sin, np.float32)
    w_q_w = np.asarray(w_q_w, np.float32)
    w_q_b = np.asarray(w_q_b, np.float32)
    w_kv_w = np.asarray(w_kv_w, np.float32)
    w_kv_b = np.asarray(w_kv_b, np.float32)
    proj_w = np.asarray(proj_w, np.float32)
    proj_b = np.asarray(proj_b, np.float32)

    xT = np.ascontiguousarray(x.reshape(BT, E).T).astype(bf16)

    cosE = np.repeat(freq_cos.T, 2, axis=0).astype(np.float32)        # [128, T]
    sinE = np.repeat(freq_sin.T, 2, axis=0).astype(np.float32)
    sinS = sinE.copy()
    sinS[0::2, :] *= -1.0                                             # even rows -sin
    cosE = cosE.astype(bf16)
    sinS = sinS.astype(bf16)

    kp = np.arange(128)[:, None]
    cc = np.arange(896)[None, :]
    maskM = (cc >= kp + 384).astype(bf16)

    ident = np.eye(128, dtype=np.float32)

    if _PROG is None:
        _PROG = _build_program()

    in_maps = []
    for c in range(NCORES):
        wq_c = np.ascontiguousarray(
            w_q_w[c * 512:(c + 1) * 512, :].T).astype(bf16)            # [E, 512]
        kT = w_kv_w[c * D:(c + 1) * D, :].T                            # [E, 128]
        vT = w_kv_w[8 * D + c * D:8 * D + (c + 1) * D, :].T
        wkv_c = np.ascontiguousarray(
            np.concatenate([kT, vT], axis=1)).astype(bf16)             # [E, 256]
        biases = np.zeros((6, 128), np.float32)
        biases[0:4, :] = w_q_b[c * 512:(c + 1) * 512].reshape(4, 128)
        biases[4, :] = w_kv_b[c * D:(c + 1) * D]
        biases[5, :] = w_kv_b[8 * D + c * D:8 * D + (c + 1) * D]
        pjt_c = np.ascontiguousarray(
            proj_w[:, c * 512:(c + 1) * 512].T).astype(bf16)           # [512, E]
        in_maps.append({
            "xT": xT, "wqT": wq_c, "wkvT": wkv_c, "biases": biases,
            "cosE": cosE, "sinS": sinS, "maskM": maskM, "projT": pjt_c,
            "ident": ident, "onescol": np.ones((128, 1), bf16),
        })

    res = run_bass_kernel_spmd(_PROG, in_maps, core_ids=list(range(NCORES)))
    out = np.zeros((BT, E), np.float32)
    for c in range(NCORES):
        out += res.results[c]["yp"].astype(np.float32)
    out = out + proj_b[None, :].astype(np.float32)
    return out.reshape(B, T, E).astype(np.float32)
